# revision 12
# baseline (speedup 1.0000x reference)
"""Trainium2 Bass kernel for nn_AttentionBlock (B=4, C=256, H=W=64, R=32).

Computes: q = Wq@skip + bq; k = Wk@gating + bk; v = Wv@skip + bv
          energy = q^T k per sample; attn = softmax(energy, axis=-1)
          out = gamma * (v @ attn^T) + skip

Sharding: 8 shards = (batch b in 0..3, m-half in 0..1). Each core handles
2048 rows (m) of the 4096x4096 attention matrix for one sample.

Per-core algorithm (matmuls in float32r = TF32-like full-rate, fp32 accum):
  - energy^T chunks [n=128, m=512] = k_chunk^T q  (n on partitions)
  - expT = exp(energy^T)  (no max subtraction; |energy| < ~10 by construction)
  - out_unnorm [m=128, C+1] = sum_n expT_chunk^T @ [vT | ones]  -- the ones
    column yields the softmax denominator for free
  - final [m, c] = (out_unnorm[:, :C] * (gamma / denom)) + skip^T  (one DVE op)
  - host transposes each core's [2048, 256] result back to [C, m] layout.
"""

import numpy as np

import concourse.bass as bass
import concourse.tile as tile
from concourse import mybir
from concourse import bass_utils

B, C, H, W = 4, 256, 64, 64
N = H * W          # 4096 spatial positions
R = C // 8         # 32 reduced dim
MH = N // 2        # 2048 m rows per core
P = 128            # partitions
NCH = N // P       # 32 n-chunks
MB = 512           # m-block (energy matmul moving width)
NMB = MH // MB     # 4 m-blocks per core
S = MB // P        # 4 m-subtiles per block
CE = C + 2         # extended cols: [v^T | ones | pad] (fp32r needs even N)

F32 = mybir.dt.float32
F32R = mybir.dt.float32r
BF16 = mybir.dt.bfloat16
AF = mybir.ActivationFunctionType
ALU = mybir.AluOpType

_WAIT_LIMIT = 1  # this walrus build supports 1 sync wait per instruction


def _r(ap):
    """View an fp32 AP as float32r (TF32-like matmul mode, same bits)."""
    return ap.bitcast(F32R)


def _split_multi_waits(nc):
    """Hoist excess per-instruction sem waits onto preceding same-engine NOPs.

    The installed walrus rejects >1 sync wait per instruction
    ("Too many sync wait commands"), while Tile freely emits several.
    """
    n_new = 0
    for f in nc.m.functions:
        for blk in f.blocks:
            insts = blk.instructions  # live list reference
            i = 0
            while i < len(insts):
                inst = insts[i]
                si = inst.sync_info
                if si is not None and len(si.on_wait) > _WAIT_LIMIT:
                    waits = list(si.on_wait)
                    si.on_wait = waits[-_WAIT_LIMIT:]
                    for j, w in enumerate(waits[:-_WAIT_LIMIT]):
                        nop = mybir.InstNoOp(
                            name=f"{inst.name}-sw{j}",
                            sync_info=mybir.SyncInfo(on_wait=[w], on_update=[]),
                            bass_nofuse=True,
                            engine=inst.engine,
                        )
                        insts.insert(i, nop)
                        i += 1
                        n_new += 1
                i += 1
    return n_new


def build_nc(split_waits=True):
    nc = bass.Bass("TRN2", target_bir_lowering=False, debug=False)

    # Per-core DRAM inputs (host pre-shards / pre-transposes)
    skf_d = nc.dram_tensor("sk_full", [C, N], BF16, kind="ExternalInput")
    skq_d = nc.dram_tensor("sk_q", [C, MH], BF16, kind="ExternalInput")
    skt_d = nc.dram_tensor("sk_t", [MH, C], F32, kind="ExternalInput")
    gt_d = nc.dram_tensor("gt_full", [C, N], BF16, kind="ExternalInput")
    wqT_d = nc.dram_tensor("wqT", [C, R], BF16, kind="ExternalInput")
    wkT_d = nc.dram_tensor("wkT", [C, R], BF16, kind="ExternalInput")
    wvT_d = nc.dram_tensor("wvT", [C, C], BF16, kind="ExternalInput")
    bq_d = nc.dram_tensor("bq2", [R, 1], F32, kind="ExternalInput")
    bk_d = nc.dram_tensor("bk2", [R, 1], F32, kind="ExternalInput")
    bve_d = nc.dram_tensor("bv_ext", [1, CE], BF16, kind="ExternalInput")
    ones_d = nc.dram_tensor("ones_h", [1, P], BF16, kind="ExternalInput")
    out_d = nc.dram_tensor("out_t", [MH, C], F32, kind="ExternalOutput")

    with tile.TileContext(nc) as tc:
        _body(nc, tc, skf_d, skq_d, skt_d, gt_d, wqT_d, wkT_d, wvT_d,
              bq_d, bk_d, bve_d, ones_d, out_d)

    if split_waits:
        _split_multi_waits(nc)
    return nc


def _body(nc, tc, skf_d, skq_d, skt_d, gt_d, wqT_d, wkT_d, wvT_d,
          bq_d, bk_d, bve_d, ones_d, out_d):
    from contextlib import ExitStack
    ctx = ExitStack()
    with ctx:
        cpool = ctx.enter_context(tc.tile_pool(name="const", bufs=1))
        bpool = ctx.enter_context(tc.tile_pool(name="big", bufs=1))
        vtpool = ctx.enter_context(tc.tile_pool(name="vt", bufs=1))
        expool = ctx.enter_context(tc.tile_pool(name="exp", bufs=4))
        smpool = ctx.enter_context(tc.tile_pool(name="small", bufs=4))
        sktpool = ctx.enter_context(tc.tile_pool(name="skt", bufs=16))
        outpool = ctx.enter_context(tc.tile_pool(name="outp", bufs=4))

        # ---- constants / weights (small DMAs first) ----
        wq0 = cpool.tile([P, R], BF16, name="wq0")
        wq1 = cpool.tile([P, R], BF16, name="wq1")
        wk0 = cpool.tile([P, R], BF16, name="wk0")
        wk1 = cpool.tile([P, R], BF16, name="wk1")
        wv0 = cpool.tile([P, C], BF16, name="wv0")
        wv1 = cpool.tile([P, C], BF16, name="wv1")
        bqs = cpool.tile([R, 1], F32, name="bqs")
        bks = cpool.tile([R, 1], F32, name="bks")
        bve = cpool.tile([1, CE], BF16, name="bve")
        ones1 = cpool.tile([1, P], BF16, name="ones1")

        nc.sync.dma_start(wq0[:], wqT_d.ap()[0:P, :])
        nc.sync.dma_start(wq1[:], wqT_d.ap()[P:C, :])
        nc.sync.dma_start(wk0[:], wkT_d.ap()[0:P, :])
        nc.sync.dma_start(wk1[:], wkT_d.ap()[P:C, :])
        nc.sync.dma_start(wv0[:], wvT_d.ap()[0:P, :])
        nc.sync.dma_start(wv1[:], wvT_d.ap()[P:C, :])
        nc.sync.dma_start(bqs[:], bq_d.ap())
        nc.sync.dma_start(bks[:], bk_d.ap())
        nc.sync.dma_start(bve[:], bve_d.ap())
        nc.sync.dma_start(ones1[:], ones_d.ap())

        # ---- big activations, split into column halves for earlier starts
        sk0 = bpool.tile([P, N], BF16, name="sk0")
        sk1 = bpool.tile([P, N], BF16, name="sk1")
        gt0 = bpool.tile([P, N], BF16, name="gt0")
        gt1 = bpool.tile([P, N], BF16, name="gt1")
        skq0 = bpool.tile([P, MH], BF16, name="skq0")
        skq1 = bpool.tile([P, MH], BF16, name="skq1")
        QH = MH // 2
        NH = N // 2
        for h in range(2):
            nc.sync.dma_start(skq0[:, h * QH:(h + 1) * QH],
                              skq_d.ap()[0:P, h * QH:(h + 1) * QH])
            nc.sync.dma_start(skq1[:, h * QH:(h + 1) * QH],
                              skq_d.ap()[P:C, h * QH:(h + 1) * QH])
        for h in range(2):
            nc.sync.dma_start(gt0[:, h * NH:(h + 1) * NH],
                              gt_d.ap()[0:P, h * NH:(h + 1) * NH])
            nc.sync.dma_start(gt1[:, h * NH:(h + 1) * NH],
                              gt_d.ap()[P:C, h * NH:(h + 1) * NH])
        for h in range(2):
            nc.sync.dma_start(sk0[:, h * NH:(h + 1) * NH],
                              skf_d.ap()[0:P, h * NH:(h + 1) * NH])
            nc.sync.dma_start(sk1[:, h * NH:(h + 1) * NH],
                              skf_d.ap()[P:C, h * NH:(h + 1) * NH])
        # skip^T tiles for the final residual add (streamed, all upfront)
        skts = []
        for t_i in range(NMB * S):
            t = sktpool.tile([P, C], F32, name=f"skt{t_i}", tag="skt")
            nc.sync.dma_start(t[:], skt_d.ap()[t_i * P:(t_i + 1) * P, :])
            skts.append(t)

        qsb = bpool.tile([R, MH], BF16, name="qsb")
        ksb = bpool.tile([R, N], BF16, name="ksb")

        with tc.tile_pool(name="p0psum", bufs=2, space="PSUM") as p0psum:
            # q = WqT^T @ skip_q + bq   -> [R, MH]
            for mb in range(NMB):
                psq = p0psum.tile([R, MB], F32, name=f"psq{mb}", tag="psq")
                nc.tensor.matmul(psq[:], wq0[:],
                                 skq0[:, mb * MB:(mb + 1) * MB],
                                 start=True, stop=False)
                nc.tensor.matmul(psq[:], wq1[:],
                                 skq1[:, mb * MB:(mb + 1) * MB],
                                 start=False, stop=True)
                nc.scalar.add(qsb[:, mb * MB:(mb + 1) * MB], psq[:], bqs[:])

            # k = WkT^T @ gating + bk   -> [R, N]
            for nb in range(N // MB):
                psk = p0psum.tile([R, MB], F32, name=f"psk{nb}", tag="psq")
                nc.tensor.matmul(psk[:], wk0[:],
                                 gt0[:, nb * MB:(nb + 1) * MB],
                                 start=True, stop=False)
                nc.tensor.matmul(psk[:], wk1[:],
                                 gt1[:, nb * MB:(nb + 1) * MB],
                                 start=False, stop=True)
                nc.scalar.add(ksb[:, nb * MB:(nb + 1) * MB], psk[:], bks[:])

            # vT_ext chunks [n=128, CE]: rows of v^T plus ones column.
            vts = []
            for j in range(NCH):
                psv = p0psum.tile([P, CE], F32, name=f"psv{j}", tag="psv")
                nc.tensor.matmul(psv[:], ones1[:], bve[:],
                                 start=True, stop=False, skip_group_check=True)
                nc.tensor.matmul(psv[:, 0:C], sk0[:, j * P:(j + 1) * P],
                                 wv0[:],
                                 start=False, stop=False, skip_group_check=True)
                nc.tensor.matmul(psv[:, 0:C], sk1[:, j * P:(j + 1) * P],
                                 wv1[:],
                                 start=False, stop=True, skip_group_check=True)
                vt = vtpool.tile([P, CE], BF16, name=f"vt{j}", tag=f"vt{j}")
                nc.vector.tensor_copy(vt[:], psv[:])
                vts.append(vt)

        # ---- main attention loop (chunk pairs share one exp) ----
        NPAIR = NCH // 2
        with tc.tile_pool(name="mpsum", bufs=1, space="PSUM") as mpsum:
            for mb in range(NMB):
                mof = mb * MB
                psum_os = [
                    mpsum.tile([P, CE], F32, name=f"po{mb}_{s}", tag="po",
                               bufs=S)
                    for s in range(S)
                ]

                def emit_energy_pair(g, mb=mb, mof=mof):
                    pe2 = mpsum.tile([P, 2 * MB], F32, name=f"pe{mb}_{g}",
                                     tag="pe2", bufs=2)
                    for jj in range(2):
                        j = 2 * g + jj
                        nc.tensor.matmul(pe2[:, jj * MB:(jj + 1) * MB],
                                         ksb[:, j * P:(j + 1) * P],
                                         qsb[:, mof:mof + MB],
                                         start=True, stop=True)
                    ex2 = expool.tile([P, 2 * MB], BF16, name=f"ex{mb}_{g}",
                                      tag="ex")
                    nc.scalar.activation(ex2[:], pe2[:], AF.Exp)
                    return ex2

                def emit_out_pair(g, ex2, psum_os=psum_os):
                    for jj in range(2):
                        j = 2 * g + jj
                        for s in range(S):
                            nc.tensor.matmul(
                                psum_os[s][:],
                                ex2[:, jj * MB + s * P:jj * MB + (s + 1) * P],
                                vts[j][:],
                                start=(j == 0), stop=(j == NCH - 1),
                                skip_group_check=True)

                prev = emit_energy_pair(0)
                for g in range(1, NPAIR):
                    cur = emit_energy_pair(g)
                    emit_out_pair(g - 1, prev)
                    prev = cur
                emit_out_pair(NPAIR - 1, prev)

                for s in range(S):
                    po = psum_os[s]
                    rc = smpool.tile([P, 1], F32, name=f"rc{mb}_{s}", tag="rc")
                    nc.vector.reciprocal(rc[:], po[:, C:C + 1])
                    ob = outpool.tile([P, C], F32, name=f"ob{mb}_{s}", tag="ob")
                    nc.vector.scalar_tensor_tensor(
                        ob[:], po[:, 0:C], rc[:], skts[mb * S + s][:],
                        ALU.mult, ALU.add)
                    row = (mb * S + s) * P
                    nc.sync.dma_start(out_d.ap()[row:row + P, :], ob[:])


_NC_CACHE = None


def _get_nc():
    global _NC_CACHE
    if _NC_CACHE is None:
        _NC_CACHE = build_nc()
    return _NC_CACHE


def make_in_maps(skip, gating, Wq, bq, Wk, bk, Wv, bv, gamma):
    import ml_dtypes
    bf16 = ml_dtypes.bfloat16
    skip = np.ascontiguousarray(np.asarray(skip, np.float32))
    gating = np.ascontiguousarray(np.asarray(gating, np.float32))
    Wq = np.asarray(Wq, np.float32)
    Wk = np.asarray(Wk, np.float32)
    Wv = np.asarray(Wv, np.float32)
    bq = np.asarray(bq, np.float32)
    bk = np.asarray(bk, np.float32)
    bv = np.asarray(bv, np.float32)
    gamma = np.asarray(gamma, np.float32)

    wqT = np.ascontiguousarray(Wq.T.astype(bf16))
    wkT = np.ascontiguousarray(Wk.T.astype(bf16))
    g = float(gamma.reshape(-1)[0])
    wvT = np.ascontiguousarray((g * Wv).T.astype(bf16))
    bq2 = np.ascontiguousarray(bq.reshape(R, 1))
    bk2 = np.ascontiguousarray(bk.reshape(R, 1))
    bv_ext = np.concatenate([g * bv, np.ones(1, np.float32), np.zeros(1, np.float32)]).reshape(1, CE)
    bv_ext = np.ascontiguousarray(bv_ext.astype(bf16))
    ones_h = np.ones((1, P), bf16)

    in_maps = []
    for s in range(8):
        b, half = divmod(s, 2)
        m0 = half * MH
        skf = skip[b].reshape(C, N)
        gtf = gating[b].reshape(C, N)
        skf_b = skf.astype(bf16)
        gtf_b = gtf.astype(bf16)
        in_maps.append({
            "sk_full": np.ascontiguousarray(skf_b),
            "sk_q": np.ascontiguousarray(skf_b[:, m0:m0 + MH]),
            "sk_t": np.ascontiguousarray(skf[:, m0:m0 + MH].T),
            "gt_full": np.ascontiguousarray(gtf_b),
            "wqT": wqT, "wkT": wkT, "wvT": wvT,
            "bq2": bq2, "bk2": bk2, "bv_ext": bv_ext,
            "ones_h": ones_h,
        })
    return in_maps


def gather_outputs(results):
    out = np.empty((B, C, H, W), np.float32)
    outf = out.reshape(B, C, N)
    for s in range(8):
        b, half = divmod(s, 2)
        m0 = half * MH
        outf[b, :, m0:m0 + MH] = results[s]["out_t"].T
    return out


def kernel(skip, gating, Wq, bq, Wk, bk, Wv, bv, gamma, **run_kwargs):
    in_maps = make_in_maps(skip, gating, Wq, bq, Wk, bk, Wv, bv, gamma)
    nc = _get_nc()
    res = bass_utils.run_bass_kernel_spmd(
        nc, in_maps, core_ids=list(range(8)), **run_kwargs)
    out = gather_outputs(res.results)
    if run_kwargs:
        return out, res
    return out
